# revision 28
# baseline (speedup 1.0000x reference)
"""Trainium2 Bass kernel: batched multi-head self-attention (nn_Attention).

y = softmax(q k^T / sqrt(64)) v, projected; x (8, 1025, 768), 12 heads x 64.

Strategy: batch-parallel across the 8 NeuronCores (one batch element per
core, no collectives). Per core, everything is kept feature-major
(transposed) so no on-chip transposes are needed:
  qkT = wqkT.T @ xT;  v = xT.T @ wvT (with a per-head ones column);
  scoresT = kT.T @ qT (keys on partitions, head pairs row-packed in the PE
  array);  exp on the scalar engine;  [v|1](128-wide).T @ attnT accumulated
  over key tiles yields the weighted values AND the softmax denominator in
  one PSUM accumulation;  normalize via fast-approx reciprocal + gpsimd
  partition-broadcast;  yT = wpT.T @ aoT + bp.

Schedule: the attention window is ACT(exp)-bound, so per (chunk, pair) all
score matmuls are emitted before all AV matmuls (exp pipelines in the gap),
and out-projection slices of completed chunks are injected between pairs as
PE filler. The 8-wide q tail (token 1024) runs first so its slow small-op
normalize overlaps the main attention instead of gating the kernel tail.

Operands are fp16 (inputs/weights/q/k/v, ~2e-3 relative accuracy) except the
exp'd attention weights, which are bf16 (exp reaches ~5e6, beyond fp16
range); all accumulation is fp32 in PSUM.
"""
import sys

try:
    import concourse.bass  # noqa: F401
except ImportError:
    sys.path.insert(0, "/opt/trn_rl_repo")

import numpy as np

from contextlib import ExitStack

import concourse.bass as bass
import concourse.tile as tile
from concourse import bacc, mybir

F32 = mybir.dt.float32
F32R = mybir.dt.float32r
BF16 = mybir.dt.bfloat16
F16 = mybir.dt.float16

C = 768
H = 12
D = 64
NTOK = 1025
T = 1032
CT = C // 128
SCALE = D ** -0.5

KT = [(i * 128, 128) for i in range(8)] + [(1024, 8)]
QC = [(0, 512), (512, 512), (1024, 8)]
VC = [(0, 512), (512, 256)]
VW = 65

B_OT_ORDER = [0, 6, 1, 7, 2, 8, 3, 9, 4, 10, 5, 11]


def build(matmul_dtype="fp16"):
    if matmul_dtype == "bf16":
        MT = AT = ATTN = BF16
    elif matmul_dtype == "fp16":
        MT = AT = F16
        ATTN = BF16
    elif matmul_dtype == "f32":
        MT = AT = ATTN = F32
    else:
        MT = AT = ATTN = F32R
    nc = bacc.Bacc("TRN2", target_bir_lowering=False, debug=False, num_devices=8)

    xT_d = nc.dram_tensor("xT", [C, T], MT, kind="ExternalInput")
    wqkT_d = nc.dram_tensor("wqkT", [C, 2 * C], MT, kind="ExternalInput")
    wvT_d = nc.dram_tensor("wvT", [C, C], MT, kind="ExternalInput")
    wpT_d = nc.dram_tensor("wpT", [C, C], MT, kind="ExternalInput")
    bp_d = nc.dram_tensor("bp", [C, 1], F32, kind="ExternalInput")
    yT_d = nc.dram_tensor("yT", [C, T], F16 if matmul_dtype == "fp16" else F32,
                          kind="ExternalOutput")

    with tile.TileContext(nc) as tc, ExitStack() as ctx:
        p_qk = ctx.enter_context(tc.tile_pool(name="qk", bufs=1))
        p_v = ctx.enter_context(tc.tile_pool(name="v", bufs=1))
        p_ao = ctx.enter_context(tc.tile_pool(name="ao", bufs=1))
        p_bp = ctx.enter_context(tc.tile_pool(name="bp", bufs=1))
        p_attn = ctx.enter_context(tc.tile_pool(name="attn", bufs=1))
        p_sm = ctx.enter_context(tc.tile_pool(name="sm", bufs=6))
        p_stage = ctx.enter_context(tc.tile_pool(name="stage", bufs=4))
        p_w = ctx.enter_context(tc.tile_pool(name="w", bufs=1))

        qkT = [p_qk.tile([128, T], AT, tag=f"qkT{i}", name=f"qkT{i}") for i in range(12)]
        v_ext = [p_v.tile([128, H * VW + 63], AT, tag=f"v{i}", name=f"v{i}") for i in range(9)]
        aoT = [p_ao.tile([128, T], MT, tag=f"ao{i}", name=f"ao{i}") for i in range(CT)]
        bp_sb = [p_bp.tile([128, 1], F32, tag=f"bp{i}", name=f"bp{i}") for i in range(CT)]

        xT = [p_w.tile([128, T], MT, tag=f"x{i}", name=f"x{i}") for i in range(CT)]
        wvT = [p_w.tile([128, C], MT, tag=f"wv{i}", name=f"wv{i}") for i in range(CT)]
        wqk = [p_w.tile([128, 2 * C], MT, tag=f"wqk{i}", name=f"wqk{i}") for i in range(CT)]
        wp = [p_w.tile([128, C], MT, tag=f"wp{i}", name=f"wp{i}") for i in range(CT)]

        # ---- input DMAs, consumption-ordered, split for queue parallelism ----
        for c in range(CT):
            nc.sync.dma_start(xT[c][:, 0:512], xT_d.ap()[c * 128:(c + 1) * 128, 0:512])
            nc.sync.dma_start(wvT[c][:, 0:512], wvT_d.ap()[c * 128:(c + 1) * 128, 0:512])
        for c in range(CT):
            nc.sync.dma_start(wvT[c][:, 512:768], wvT_d.ap()[c * 128:(c + 1) * 128, 512:768])
            nc.sync.dma_start(xT[c][:, 512:T], xT_d.ap()[c * 128:(c + 1) * 128, 512:T])
        for c in range(CT):
            nc.sync.dma_start(wqk[c][:, 0:C], wqkT_d.ap()[c * 128:(c + 1) * 128, 0:C])
        for c in range(CT):
            nc.sync.dma_start(wqk[c][:, C:2 * C], wqkT_d.ap()[c * 128:(c + 1) * 128, C:2 * C])
        for c in range(CT):
            nc.sync.dma_start(wp[c][:], wpT_d.ap()[c * 128:(c + 1) * 128, :])
            nc.sync.dma_start(bp_sb[c][:], bp_d.ap()[c * 128:(c + 1) * 128, :])
        # softmax denominators: heads 4g..4g+3 live in den_g/rec_g at
        # 32-aligned rows 0/32/64/96 (engine APs only lower correctly at
        # partition bases 0/32/64/96); gathered by DMA, reciprocal'd in one
        # batched [128, qsz] DVE op per 2 pairs (reciprocal cost is
        # free-dim-bound, so the junk rows are free).
        den_g = [p_bp.tile([128, T], F32, tag=f"den{g}", name=f"den{g}")
                 for g in range(3)]
        rec_g = [p_bp.tile([128, T], F32, tag=f"rec{g}", name=f"rec{g}")
                 for g in range(3)]
        for g in range(3):
            nc.vector.memset(den_g[g][:], 1.0)

        # ---- phase C: v projection (c-outer within nt-chunks for DMA overlap) ----
        with nc.named_scope("v_proj"), \
             tc.tile_pool(name="psC", bufs=6, space="PSUM") as psC:
            for chunk in ((0, 1, 2), (3, 4, 5), (6, 7, 8)):
                for (voff, vsz) in VC:
                    pss = {}
                    for nt in chunk:
                        pss[nt] = psC.tile([128, 512], F32, tag="vproj", name="ps_v")
                    for c in range(CT):
                        for nt in chunk:
                            noff, nsz = KT[nt]
                            nc.tensor.matmul(
                                pss[nt][:nsz, :vsz],
                                xT[c][:, noff:noff + nsz],
                                wvT[c][:, voff:voff + vsz],
                                start=(c == 0), stop=(c == CT - 1),
                            )
                    for nt in chunk:
                        noff, nsz = KT[nt]
                        nh = vsz // D
                        h0 = voff // D
                        dst = (
                            v_ext[nt][0:nsz, h0 * VW:(h0 + nh) * VW]
                            .rearrange("p (hh w) -> p hh w", w=VW)[:, :, 0:D]
                        )
                        src = pss[nt][0:nsz, 0:vsz].rearrange("p (hh w) -> p hh w", w=D)
                        nc.vector.tensor_copy(dst, src)
                for nt in chunk:
                    noff, nsz = KT[nt]
                    # ones column (valid tokens only) + zeroed pad/tail
                    if nt < 8:
                        ones_col = (
                            v_ext[nt][0:nsz, 0:H * VW]
                            .rearrange("p (hh w) -> p hh w", w=VW)[:, :, D:VW]
                        )
                        _memset(nc, AT, ones_col, one=True)
                    else:
                        pad_col = (
                            v_ext[nt][0:nsz, 0:H * VW]
                            .rearrange("p (hh w) -> p hh w", w=VW)[:, :, D:VW]
                        )
                        _memset(nc, AT, pad_col, one=False)
                        one_row = (
                            v_ext[nt][0:1, 0:H * VW]
                            .rearrange("p (hh w) -> p hh w", w=VW)[:, :, D:VW]
                        )
                        _memset(nc, AT, one_row, one=True)
                    _memset(nc, AT, v_ext[nt][:, H * VW:H * VW + 63], one=False)

        # ---- phase B: q/k projection (pair (5,11) deferred into attention) ----
        def b_otile(psp, tag, ot, bufs=3):
            for (qoff, qsz) in QC[:2]:
                ps = psp.tile([128, 512], F32, tag=tag, name="ps_proj", bufs=bufs)
                for c in range(CT):
                    nc.tensor.matmul(
                        ps[:, :qsz],
                        wqk[c][:, ot * 128:(ot + 1) * 128],
                        xT[c][:, qoff:qoff + qsz],
                        start=(c == 0), stop=(c == CT - 1),
                    )
                nc.vector.tensor_copy(qkT[ot][:, qoff:qoff + qsz], ps[:, :qsz])

        def b_tailpair(psp, tag, pair_ot, bufs=3):
            qoff, qsz = QC[2]
            ps = psp.tile([128, 512], F32, tag=tag, name="ps_proj", bufs=bufs)
            for oi, ot in enumerate((pair_ot, 6 + pair_ot)):
                for c in range(CT):
                    nc.tensor.matmul(
                        ps[:, oi * 8:oi * 8 + qsz],
                        wqk[c][:, ot * 128:(ot + 1) * 128],
                        xT[c][:, qoff:qoff + qsz],
                        start=(c == 0), stop=(c == CT - 1),
                    )
            nc.vector.tensor_copy(qkT[pair_ot][:, qoff:qoff + qsz], ps[:, 0:qsz])
            nc.vector.tensor_copy(qkT[6 + pair_ot][:, qoff:qoff + qsz],
                                  ps[:, 8:8 + qsz])

        with nc.named_scope("qk_proj"), \
             tc.tile_pool(name="psB", bufs=3, space="PSUM") as psB:
            for ot in B_OT_ORDER:
                if ot in (5, 11):
                    continue
                b_otile(psB, "proj", ot)
            for pair_ot in range(5):
                b_tailpair(psB, "proj", pair_ot)

        # ---- attention + out-projection, software-pipelined ----
        with nc.named_scope("attn"), \
             tc.tile_pool(name="psD", bufs=1, space="PSUM") as psD:

            def e_slice(ot, qoff, qsz):
                ps = psD.tile([128, 512], F32, tag="av", name="ps_av", bufs=2)
                for c in range(CT):
                    nc.tensor.matmul(
                        ps[:, :qsz],
                        wp[c][:, ot * 128:(ot + 1) * 128],
                        aoT[c][:, qoff:qoff + qsz],
                        start=(c == 0), stop=(c == CT - 1),
                    )
                st = p_stage.tile([128, 512], F16 if MT == F16 else F32,
                                  tag="ystage", name="ystage")
                nc.vector.tensor_scalar_add(st[:, :qsz], ps[:, :qsz], bp_sb[ot][:, 0:1])
                nc.sync.dma_start(yT_d.ap()[ot * 128:(ot + 1) * 128, qoff:qoff + qsz],
                                  st[:, :qsz])

            def qc2_slice(pair, h_in_pair):
                _emit_qc2_head(nc, qkT, v_ext, aoT, psD, p_attn, p_sm, ATTN,
                               pair, h_in_pair)

            # PE filler work injected at pair boundaries of the ACT-bound
            # attention stream: (qc index, pair index) -> list of closures
            fillers = {
                (0, 0): [lambda: b_otile(psD, "av", 5, bufs=2),
                         lambda: b_otile(psD, "av", 11, bufs=2),
                         lambda: b_tailpair(psD, "av", 5, bufs=2)],
                (0, 1): [lambda: qc2_slice(0, 0), lambda: qc2_slice(0, 1),
                         lambda: qc2_slice(1, 0), lambda: qc2_slice(1, 1)],
                (0, 2): [lambda: qc2_slice(2, 0), lambda: qc2_slice(2, 1),
                         lambda: qc2_slice(3, 0), lambda: qc2_slice(3, 1)],
                (0, 3): [lambda: qc2_slice(4, 0), lambda: qc2_slice(4, 1),
                         lambda: qc2_slice(5, 0), lambda: qc2_slice(5, 1)],
                (0, 4): [lambda ot=ot: e_slice(ot, QC[2][0], QC[2][1])
                         for ot in range(3)],
                (0, 5): [lambda ot=ot: e_slice(ot, QC[2][0], QC[2][1])
                         for ot in range(3, CT)],
                (1, 0): [lambda: e_slice(0, QC[0][0], QC[0][1])],
                (1, 1): [lambda: e_slice(1, QC[0][0], QC[0][1])],
                (1, 2): [lambda: e_slice(2, QC[0][0], QC[0][1])],
                (1, 3): [lambda: e_slice(3, QC[0][0], QC[0][1])],
                (1, 4): [lambda: e_slice(4, QC[0][0], QC[0][1])],
                (1, 5): [lambda: e_slice(5, QC[0][0], QC[0][1])],
            }

            av_sb_cache = {}
            for qi, (qoff, qsz) in enumerate(QC[:2]):
                for pair in range(6):
                    h0 = 2 * pair
                    h1 = 2 * pair + 1
                    # -- scores: all 9 key tiles, both heads --
                    at = {}  # (kt, hip) -> (attn tile, col offset)
                    for g in range(4):
                        kta, ktb = 2 * g, 2 * g + 1
                        for hip in range(2):
                            sc = psD.tile([128, 1024], F32, tag="sc", name="ps_sc",
                                          bufs=3)
                            _scores_mm(nc, qkT, sc, pair, hip, kta, 0, qoff, qsz)
                            _scores_mm(nc, qkT, sc, pair, hip, ktb, 512, qoff, qsz)
                            a = p_attn.tile([128, 1024], ATTN, tag="attnT",
                                            name="attnT", bufs=9)
                            nc.scalar.activation(
                                a[:, 0:1024], sc[:, 0:1024],
                                mybir.ActivationFunctionType.Exp, scale=SCALE,
                            )
                            at[(kta, hip)] = (a, 0)
                            at[(ktb, hip)] = (a, 512)
                    # key tail (8 keys), both heads packed, one exp
                    sc = psD.tile([128, 1024], F32, tag="sc", name="ps_sc", bufs=3)
                    _scores_mm(nc, qkT, sc, pair, 0, 8, 0, qoff, qsz)
                    _scores_mm(nc, qkT, sc, pair, 1, 8, 512, qoff, qsz)
                    a = p_attn.tile([128, 1024], ATTN, tag="attnT", name="attnT",
                                    bufs=9)
                    nc.scalar.activation(
                        a[0:8, 0:1024], sc[0:8, 0:1024],
                        mybir.ActivationFunctionType.Exp, scale=SCALE,
                    )
                    at[(8, 0)] = (a, 0)
                    at[(8, 1)] = (a, 512)

                    # -- AV: accumulate over key tiles, both heads --
                    avs = {
                        h0: psD.tile([128, 512], F32, tag="av", name="ps_av", bufs=2),
                        h1: psD.tile([128, 512], F32, tag="av", name="ps_av", bufs=2),
                    }
                    for kt in range(9):
                        for hip in range(2):
                            a, aoff = at[(kt, hip)]
                            _av_mm(nc, v_ext, avs[2 * pair + hip], 2 * pair + hip,
                                   kt, a, aoff, qsz)

                    # copy out + gather denominators; batched recip per 2 pairs
                    for hip in range(2):
                        h = 2 * pair + hip
                        av_sb = p_sm.tile([128, 512], F32, tag="avsb", name="avsb")
                        av_sb_cache[h] = av_sb
                        nc.vector.tensor_copy(av_sb[0:VW, 0:qsz],
                                              avs[h][0:VW, 0:qsz])
                        # cross-partition gather: 2-partition APs at 32-aligned
                        # bases are the only forms that survive BIR lowering.
                        # Row 32m+1 receives av_sb row 65 junk (never read).
                        nc.vector.tensor_copy(
                            den_g[h // 4][32 * (h % 4):32 * (h % 4) + 2,
                                          qoff:qoff + qsz],
                            av_sb[D:D + 2, 0:qsz])
                    if pair % 2 == 1:
                        g = pair // 2
                        nc.vector.reciprocal(rec_g[g][0:128, qoff:qoff + qsz],
                                             den_g[g][0:128, qoff:qoff + qsz])
                        for h in range(4 * g, 4 * g + 4):
                            row = 32 * (h % 4)
                            # partition_broadcast only reads correctly from
                            # partition 0, so stage the head's row there first
                            rc = p_sm.tile([2, 512], F32, tag="rec0", name="rec0")
                            nc.vector.tensor_copy(
                                rc[0:2, 0:qsz],
                                rec_g[g][row:row + 2, qoff:qoff + qsz])
                            bc = p_sm.tile([64, 512], F32, tag="bc", name="bc")
                            nc.gpsimd.partition_broadcast(
                                bc[0:64, 0:qsz], rc[0:1, 0:qsz], channels=64)
                            nc.vector.tensor_mul(
                                aoT[h // 2][(h % 2) * 64:(h % 2) * 64 + 64,
                                            qoff:qoff + qsz],
                                av_sb_cache[h][0:D, 0:qsz],
                                bc[0:64, 0:qsz],
                            )

                    for f in fillers.get((qi, pair), []):
                        f()
            # remaining out-projection work
            for ot in range(CT):
                e_slice(ot, QC[1][0], QC[1][1])

    nc.compile()
    return nc


def _memset(nc, AT, ap, one):
    if AT == BF16:
        nc.vector.memset(ap.bitcast(mybir.dt.uint16), 0x3F80 if one else 0)
    elif AT == F16:
        nc.vector.memset(ap.bitcast(mybir.dt.uint16), 0x3C00 if one else 0)
    else:
        nc.vector.memset(ap.bitcast(mybir.dt.uint32), 0x3F800000 if one else 0)


def _scores_mm(nc, qkT, sc, pair, h_in_pair, kt, dst_off, qoff, qsz):
    koff, ksz = KT[kt]
    p0 = 64 * h_in_pair
    nc.tensor.matmul(
        sc[0:ksz, dst_off:dst_off + qsz],
        qkT[6 + pair][p0:p0 + 64, koff:koff + ksz],
        qkT[pair][p0:p0 + 64, qoff:qoff + qsz],
        start=True, stop=True,
    )


def _av_mm(nc, v_ext, av, h, kt, src, src_off, qsz):
    koff, ksz = KT[kt]
    nc.tensor.matmul(
        av[0:128, 0:qsz],
        v_ext[kt][0:ksz, h * VW:h * VW + 128],
        src[0:ksz, src_off:src_off + qsz],
        start=(kt == 0), stop=(kt == 8),
        skip_group_check=True,
    )


_APPROX_RECIP = False


def _normalize(nc, p_sm, aoT, avs, pair, h_in_pair, qoff, qsz):
    h = 2 * pair + h_in_pair
    p0 = 64 * h_in_pair
    av_sb = p_sm.tile([128, 512], F32, tag="avsb", name="avsb")
    nc.vector.tensor_copy(av_sb[0:VW, 0:qsz], avs[h][0:VW, 0:qsz])
    rec = p_sm.tile([1, 512], F32, tag="rec", name="rec")
    if _APPROX_RECIP:
        nc.vector.reciprocal_approx_fast(rec[0:1, 0:qsz], av_sb[D:VW, 0:qsz])
    else:
        nc.vector.reciprocal(rec[0:1, 0:qsz], av_sb[D:VW, 0:qsz])
    bc = p_sm.tile([64, 512], F32, tag="bc", name="bc")
    nc.gpsimd.partition_broadcast(bc[0:64, 0:qsz], rec[0:1, 0:qsz])
    nc.vector.tensor_mul(
        aoT[pair][p0:p0 + 64, qoff:qoff + qsz],
        av_sb[0:D, 0:qsz],
        bc[0:64, 0:qsz],
    )


def _emit_qc2_head(nc, qkT, v_ext, aoT, psD, p_attn, p_sm, ATTN, pair, h_in_pair):
    qoff, qsz = QC[2]
    h = 2 * pair + h_in_pair
    sc = psD.tile([128, 1024], F32, tag="sc", name="ps_sc", bufs=3)
    for kt in range(9):
        _scores_mm(nc, qkT, sc, pair, h_in_pair, kt, kt * 8, qoff, qsz)
    a = p_attn.tile([128, 1024], ATTN, tag="attnT", name="attnT", bufs=9)
    nc.scalar.activation(
        a[:, 0:64], sc[:, 0:64],
        mybir.ActivationFunctionType.Exp, scale=SCALE,
    )
    nc.scalar.activation(
        a[0:8, 64:72], sc[0:8, 64:72],
        mybir.ActivationFunctionType.Exp, scale=SCALE,
    )
    av = psD.tile([128, 512], F32, tag="av", name="ps_av", bufs=2)
    for kt in range(9):
        _av_mm(nc, v_ext, av, h, kt, a, kt * 8, qsz)
    _normalize(nc, p_sm, aoT, {h: av}, pair, h_in_pair, qoff, qsz)


_NC_CACHE = {}
_MODE = "fp16"


def kernel(x, w_qkv, w_proj, b_proj):
    x = np.asarray(x, np.float32)
    w_qkv = np.asarray(w_qkv, np.float32)
    w_proj = np.asarray(w_proj, np.float32)
    b_proj = np.asarray(b_proj, np.float32)
    B = x.shape[0]
    assert x.shape == (8, NTOK, C), x.shape

    mt = np.float16 if _MODE == "fp16" else np.float32
    wqkT = np.ascontiguousarray(w_qkv[:2 * C].T.astype(mt))
    wvT = np.ascontiguousarray(w_qkv[2 * C:].T.astype(mt))
    wpT = np.ascontiguousarray(w_proj.T.astype(mt))
    bp = np.ascontiguousarray(b_proj.reshape(C, 1))
    in_maps = []
    for b in range(B):
        xT = np.zeros((C, T), mt)
        xT[:, :NTOK] = x[b].T.astype(mt)
        in_maps.append({"xT": xT, "wqkT": wqkT, "wvT": wvT, "wpT": wpT, "bp": bp})

    if _MODE not in _NC_CACHE:
        _NC_CACHE[_MODE] = build(matmul_dtype=_MODE)
    nc = _NC_CACHE[_MODE]
    from concourse import bass_utils
    res = bass_utils.run_bass_kernel_spmd(nc, in_maps, core_ids=list(range(B)),
                                          trace=False)
    y = np.stack([res.results[b]["yT"][:, :NTOK].T for b in range(B)])
    return np.ascontiguousarray(y.astype(np.float32))


# revision 30
# speedup vs baseline: 1.1824x; 1.1824x over previous
"""Trainium2 Bass kernel: batched multi-head self-attention (nn_Attention).

y = softmax(q k^T / sqrt(64)) v, projected; x (8, 1025, 768), 12 heads x 64.

Strategy: batch-parallel across the 8 NeuronCores (one batch element per
core, no collectives). Per core, everything is kept feature-major
(transposed) so no on-chip transposes are needed:
  qkT = wqkT.T @ xT;  v = xT.T @ wvT (with a per-head ones column);
  scoresT = kT.T @ qT (keys on partitions, head pairs row-packed in the PE
  array);  exp on the scalar engine;  [v|1](128-wide).T @ attnT accumulated
  over key tiles yields the weighted values AND the softmax denominator in
  one PSUM accumulation;  normalize via fast-approx reciprocal + gpsimd
  partition-broadcast;  yT = wpT.T @ aoT + bp.

Schedule: the attention window is ACT(exp)-bound, so per (chunk, pair) all
score matmuls are emitted before all AV matmuls (exp pipelines in the gap),
and out-projection slices of completed chunks are injected between pairs as
PE filler. The 8-wide q tail (token 1024) runs first so its slow small-op
normalize overlaps the main attention instead of gating the kernel tail.

Operands are fp16 (inputs/weights/q/k/v, ~2e-3 relative accuracy) except the
exp'd attention weights, which are bf16 (exp reaches ~5e6, beyond fp16
range); all accumulation is fp32 in PSUM.
"""
import sys

try:
    import concourse.bass  # noqa: F401
except ImportError:
    sys.path.insert(0, "/opt/trn_rl_repo")

import numpy as np

from contextlib import ExitStack

import concourse.bass as bass
import concourse.tile as tile
from concourse import bacc, mybir

F32 = mybir.dt.float32
F32R = mybir.dt.float32r
BF16 = mybir.dt.bfloat16
F16 = mybir.dt.float16

C = 768
H = 12
D = 64
NTOK = 1025
T = 1032
CT = C // 128
SCALE = D ** -0.5

KT = [(i * 128, 128) for i in range(8)] + [(1024, 8)]
QC = [(0, 512), (512, 512), (1024, 8)]
VC = [(0, 512), (512, 256)]
VW = 65

B_OT_ORDER = [0, 6, 1, 7, 2, 8, 3, 9, 4, 10, 5, 11]


def build(matmul_dtype="fp16"):
    if matmul_dtype == "bf16":
        MT = AT = ATTN = BF16
    elif matmul_dtype == "fp16":
        MT = AT = F16
        ATTN = BF16
    elif matmul_dtype == "f32":
        MT = AT = ATTN = F32
    else:
        MT = AT = ATTN = F32R
    nc = bacc.Bacc("TRN2", target_bir_lowering=False, debug=False, num_devices=8)

    xT_d = nc.dram_tensor("xT", [C, T], MT, kind="ExternalInput")
    wqkT_d = nc.dram_tensor("wqkT", [C, 2 * C], MT, kind="ExternalInput")
    wvT_d = nc.dram_tensor("wvT", [C, C], MT, kind="ExternalInput")
    wpT_d = nc.dram_tensor("wpT", [C, C], MT, kind="ExternalInput")
    bp_d = nc.dram_tensor("bp", [C, 1], F32, kind="ExternalInput")
    yT_d = nc.dram_tensor("yT", [C, T], F16 if matmul_dtype == "fp16" else F32,
                          kind="ExternalOutput")

    with tile.TileContext(nc) as tc, ExitStack() as ctx:
        p_qk = ctx.enter_context(tc.tile_pool(name="qk", bufs=1))
        p_v = ctx.enter_context(tc.tile_pool(name="v", bufs=1))
        p_ao = ctx.enter_context(tc.tile_pool(name="ao", bufs=1))
        p_bp = ctx.enter_context(tc.tile_pool(name="bp", bufs=1))
        p_attn = ctx.enter_context(tc.tile_pool(name="attn", bufs=1))
        p_sm = ctx.enter_context(tc.tile_pool(name="sm", bufs=6))
        p_stage = ctx.enter_context(tc.tile_pool(name="stage", bufs=4))
        p_w = ctx.enter_context(tc.tile_pool(name="w", bufs=1))

        qkT = [p_qk.tile([128, T], AT, tag=f"qkT{i}", name=f"qkT{i}") for i in range(12)]
        v_ext = [p_v.tile([128, H * VW + 63], AT, tag=f"v{i}", name=f"v{i}") for i in range(9)]
        aoT = [p_ao.tile([128, T], MT, tag=f"ao{i}", name=f"ao{i}") for i in range(CT)]
        bp_sb = [p_bp.tile([128, 1], F32, tag=f"bp{i}", name=f"bp{i}") for i in range(CT)]

        xT = [p_w.tile([128, T], MT, tag=f"x{i}", name=f"x{i}") for i in range(CT)]
        wvT = [p_w.tile([128, C], MT, tag=f"wv{i}", name=f"wv{i}") for i in range(CT)]
        wqk = [p_w.tile([128, 2 * C], MT, tag=f"wqk{i}", name=f"wqk{i}") for i in range(CT)]
        wp = [p_w.tile([128, C], MT, tag=f"wp{i}", name=f"wp{i}") for i in range(CT)]

        # ---- input DMAs, consumption-ordered, split for queue parallelism ----
        for c in range(CT):
            nc.sync.dma_start(xT[c][:, 0:512], xT_d.ap()[c * 128:(c + 1) * 128, 0:512])
            nc.sync.dma_start(wvT[c][:, 0:512], wvT_d.ap()[c * 128:(c + 1) * 128, 0:512])
        for c in range(CT):
            nc.sync.dma_start(wvT[c][:, 512:768], wvT_d.ap()[c * 128:(c + 1) * 128, 512:768])
            nc.sync.dma_start(xT[c][:, 512:T], xT_d.ap()[c * 128:(c + 1) * 128, 512:T])
        for c in range(CT):
            nc.sync.dma_start(wqk[c][:, 0:C], wqkT_d.ap()[c * 128:(c + 1) * 128, 0:C])
        for c in range(CT):
            nc.sync.dma_start(wqk[c][:, C:2 * C], wqkT_d.ap()[c * 128:(c + 1) * 128, C:2 * C])
        for c in range(CT):
            nc.sync.dma_start(wp[c][:], wpT_d.ap()[c * 128:(c + 1) * 128, :])
            nc.sync.dma_start(bp_sb[c][:], bp_d.ap()[c * 128:(c + 1) * 128, :])
        # softmax denominators: heads 4g..4g+3 live in den_g/rec_g at
        # 32-aligned rows 0/32/64/96 (engine APs only lower correctly at
        # partition bases 0/32/64/96); gathered by DMA, reciprocal'd in one
        # batched [128, qsz] DVE op per 2 pairs (reciprocal cost is
        # free-dim-bound, so the junk rows are free).
        den_g = [p_bp.tile([128, T], F32, tag=f"den{g}", name=f"den{g}")
                 for g in range(3)]
        rec_g = [p_bp.tile([128, T], F32, tag=f"rec{g}", name=f"rec{g}")
                 for g in range(3)]
        for g in range(3):
            nc.vector.memset(den_g[g][:], 1.0)

        # ---- phase C: v projection (c-outer within nt-chunks for DMA overlap) ----
        with nc.named_scope("v_proj"), \
             tc.tile_pool(name="psC", bufs=6, space="PSUM") as psC:
            for chunk in ((0, 1, 2), (3, 4, 5), (6, 7, 8)):
                for (voff, vsz) in VC:
                    pss = {}
                    for nt in chunk:
                        pss[nt] = psC.tile([128, 512], F32, tag="vproj", name="ps_v")
                    for c in range(CT):
                        for nt in chunk:
                            noff, nsz = KT[nt]
                            nc.tensor.matmul(
                                pss[nt][:nsz, :vsz],
                                xT[c][:, noff:noff + nsz],
                                wvT[c][:, voff:voff + vsz],
                                start=(c == 0), stop=(c == CT - 1),
                            )
                    for nt in chunk:
                        noff, nsz = KT[nt]
                        nh = vsz // D
                        h0 = voff // D
                        dst = (
                            v_ext[nt][0:nsz, h0 * VW:(h0 + nh) * VW]
                            .rearrange("p (hh w) -> p hh w", w=VW)[:, :, 0:D]
                        )
                        src = pss[nt][0:nsz, 0:vsz].rearrange("p (hh w) -> p hh w", w=D)
                        nc.vector.tensor_copy(dst, src)
                for nt in chunk:
                    noff, nsz = KT[nt]
                    # ones column (valid tokens only) + zeroed pad/tail
                    if nt < 8:
                        ones_col = (
                            v_ext[nt][0:nsz, 0:H * VW]
                            .rearrange("p (hh w) -> p hh w", w=VW)[:, :, D:VW]
                        )
                        _memset(nc, AT, ones_col, one=True)
                    else:
                        pad_col = (
                            v_ext[nt][0:nsz, 0:H * VW]
                            .rearrange("p (hh w) -> p hh w", w=VW)[:, :, D:VW]
                        )
                        _memset(nc, AT, pad_col, one=False)
                        one_row = (
                            v_ext[nt][0:1, 0:H * VW]
                            .rearrange("p (hh w) -> p hh w", w=VW)[:, :, D:VW]
                        )
                        _memset(nc, AT, one_row, one=True)
                    _memset(nc, AT, v_ext[nt][:, H * VW:H * VW + 63], one=False)

        # ---- phase B: q/k projection (pair (5,11) deferred into attention) ----
        def b_otile(psp, tag, ot, bufs=3):
            for (qoff, qsz) in QC[:2]:
                ps = psp.tile([128, 512], F32, tag=tag, name="ps_proj", bufs=bufs)
                for c in range(CT):
                    nc.tensor.matmul(
                        ps[:, :qsz],
                        wqk[c][:, ot * 128:(ot + 1) * 128],
                        xT[c][:, qoff:qoff + qsz],
                        start=(c == 0), stop=(c == CT - 1),
                    )
                nc.vector.tensor_copy(qkT[ot][:, qoff:qoff + qsz], ps[:, :qsz])

        def b_tailpair(psp, tag, pair_ot, bufs=3):
            qoff, qsz = QC[2]
            ps = psp.tile([128, 512], F32, tag=tag, name="ps_proj", bufs=bufs)
            for oi, ot in enumerate((pair_ot, 6 + pair_ot)):
                for c in range(CT):
                    nc.tensor.matmul(
                        ps[:, oi * 8:oi * 8 + qsz],
                        wqk[c][:, ot * 128:(ot + 1) * 128],
                        xT[c][:, qoff:qoff + qsz],
                        start=(c == 0), stop=(c == CT - 1),
                    )
            nc.vector.tensor_copy(qkT[pair_ot][:, qoff:qoff + qsz], ps[:, 0:qsz])
            nc.vector.tensor_copy(qkT[6 + pair_ot][:, qoff:qoff + qsz],
                                  ps[:, 8:8 + qsz])

        with nc.named_scope("qk_proj"), \
             tc.tile_pool(name="psB", bufs=3, space="PSUM") as psB:
            for ot in B_OT_ORDER:
                b_otile(psB, "proj", ot)
            for pair_ot in range(CT):
                b_tailpair(psB, "proj", pair_ot)

        # ---- attention + out-projection, software-pipelined ----
        with nc.named_scope("attn"), \
             tc.tile_pool(name="psD", bufs=1, space="PSUM") as psD:

            def e_slice(ot, qoff, qsz):
                ps = psD.tile([128, 512], F32, tag="av", name="ps_av", bufs=2)
                for c in range(CT):
                    nc.tensor.matmul(
                        ps[:, :qsz],
                        wp[c][:, ot * 128:(ot + 1) * 128],
                        aoT[c][:, qoff:qoff + qsz],
                        start=(c == 0), stop=(c == CT - 1),
                    )
                st = p_stage.tile([128, 512], F16 if MT == F16 else F32,
                                  tag="ystage", name="ystage")
                nc.vector.tensor_scalar_add(st[:, :qsz], ps[:, :qsz], bp_sb[ot][:, 0:1])
                nc.sync.dma_start(yT_d.ap()[ot * 128:(ot + 1) * 128, qoff:qoff + qsz],
                                  st[:, :qsz])

            def qc2_slice(pair, h_in_pair):
                _emit_qc2_head(nc, qkT, v_ext, aoT, psD, p_attn, p_sm, ATTN,
                               pair, h_in_pair)

            # q-tail attention first: its slow small-op normalize overlaps
            # the main attention instead of gating the kernel tail
            for p2 in range(6):
                for hip2 in range(2):
                    qc2_slice(p2, hip2)

            # PE filler work injected at pair boundaries of the ACT-bound
            # attention stream: (qc index, pair index) -> list of closures
            fillers = {
                (0, 0): [lambda ot=ot: e_slice(ot, QC[2][0], QC[2][1])
                         for ot in range(CT)],
                (1, 0): [lambda: e_slice(0, QC[0][0], QC[0][1])],
                (1, 1): [lambda: e_slice(1, QC[0][0], QC[0][1])],
                (1, 2): [lambda: e_slice(2, QC[0][0], QC[0][1])],
                (1, 3): [lambda: e_slice(3, QC[0][0], QC[0][1])],
                (1, 4): [lambda: e_slice(4, QC[0][0], QC[0][1])],
                (1, 5): [lambda: e_slice(5, QC[0][0], QC[0][1])],
            }

            av_sb_cache = {}
            for qi, (qoff, qsz) in enumerate(QC[:2]):
                for pair in range(6):
                    h0 = 2 * pair
                    h1 = 2 * pair + 1
                    # -- scores: all 9 key tiles, both heads --
                    at = {}  # (kt, hip) -> (attn tile, col offset)
                    for g in range(4):
                        kta, ktb = 2 * g, 2 * g + 1
                        for hip in range(2):
                            sc = psD.tile([128, 1024], F32, tag="sc", name="ps_sc",
                                          bufs=3)
                            _scores_mm(nc, qkT, sc, pair, hip, kta, 0, qoff, qsz)
                            _scores_mm(nc, qkT, sc, pair, hip, ktb, 512, qoff, qsz)
                            a = p_attn.tile([128, 1024], ATTN, tag="attnT",
                                            name="attnT", bufs=9)
                            nc.scalar.activation(
                                a[:, 0:1024], sc[:, 0:1024],
                                mybir.ActivationFunctionType.Exp, scale=SCALE,
                            )
                            at[(kta, hip)] = (a, 0)
                            at[(ktb, hip)] = (a, 512)
                    # key tail (8 keys), both heads packed, one exp
                    sc = psD.tile([128, 1024], F32, tag="sc", name="ps_sc", bufs=3)
                    _scores_mm(nc, qkT, sc, pair, 0, 8, 0, qoff, qsz)
                    _scores_mm(nc, qkT, sc, pair, 1, 8, 512, qoff, qsz)
                    a = p_attn.tile([128, 1024], ATTN, tag="attnT", name="attnT",
                                    bufs=9)
                    nc.scalar.activation(
                        a[0:8, 0:1024], sc[0:8, 0:1024],
                        mybir.ActivationFunctionType.Exp, scale=SCALE,
                    )
                    at[(8, 0)] = (a, 0)
                    at[(8, 1)] = (a, 512)

                    # -- AV: accumulate over key tiles, both heads --
                    avs = {
                        h0: psD.tile([128, 512], F32, tag="av", name="ps_av", bufs=2),
                        h1: psD.tile([128, 512], F32, tag="av", name="ps_av", bufs=2),
                    }
                    for kt in range(9):
                        for hip in range(2):
                            a, aoff = at[(kt, hip)]
                            _av_mm(nc, v_ext, avs[2 * pair + hip], 2 * pair + hip,
                                   kt, a, aoff, qsz)

                    # copy out + gather denominators; batched recip per 2 pairs
                    for hip in range(2):
                        h = 2 * pair + hip
                        av_sb = p_sm.tile([128, 512], F32, tag="avsb", name="avsb")
                        av_sb_cache[h] = av_sb
                        nc.vector.tensor_copy(av_sb[0:VW, 0:qsz],
                                              avs[h][0:VW, 0:qsz])
                        # cross-partition gather: 2-partition APs at 32-aligned
                        # bases are the only forms that survive BIR lowering.
                        # Row 32m+1 receives av_sb row 65 junk (never read).
                        nc.vector.tensor_copy(
                            den_g[h // 4][32 * (h % 4):32 * (h % 4) + 2,
                                          qoff:qoff + qsz],
                            av_sb[D:D + 2, 0:qsz])
                    if pair % 2 == 1:
                        g = pair // 2
                        nc.vector.reciprocal(rec_g[g][0:128, qoff:qoff + qsz],
                                             den_g[g][0:128, qoff:qoff + qsz])
                        for h in range(4 * g, 4 * g + 4):
                            row = 32 * (h % 4)
                            # partition_broadcast only reads correctly from
                            # partition 0, so stage the head's row there first
                            rc = p_sm.tile([2, 512], F32, tag="rec0", name="rec0")
                            nc.vector.tensor_copy(
                                rc[0:2, 0:qsz],
                                rec_g[g][row:row + 2, qoff:qoff + qsz])
                            bc = p_sm.tile([64, 512], F32, tag="bc", name="bc")
                            nc.gpsimd.partition_broadcast(
                                bc[0:64, 0:qsz], rc[0:1, 0:qsz], channels=64)
                            nc.vector.tensor_mul(
                                aoT[h // 2][(h % 2) * 64:(h % 2) * 64 + 64,
                                            qoff:qoff + qsz],
                                av_sb_cache[h][0:D, 0:qsz],
                                bc[0:64, 0:qsz],
                            )

                    for f in fillers.get((qi, pair), []):
                        f()
            # remaining out-projection work
            for ot in range(CT):
                e_slice(ot, QC[1][0], QC[1][1])

    nc.compile()
    return nc


def _memset(nc, AT, ap, one):
    if AT == BF16:
        nc.vector.memset(ap.bitcast(mybir.dt.uint16), 0x3F80 if one else 0)
    elif AT == F16:
        nc.vector.memset(ap.bitcast(mybir.dt.uint16), 0x3C00 if one else 0)
    else:
        nc.vector.memset(ap.bitcast(mybir.dt.uint32), 0x3F800000 if one else 0)


def _scores_mm(nc, qkT, sc, pair, h_in_pair, kt, dst_off, qoff, qsz):
    koff, ksz = KT[kt]
    p0 = 64 * h_in_pair
    nc.tensor.matmul(
        sc[0:ksz, dst_off:dst_off + qsz],
        qkT[6 + pair][p0:p0 + 64, koff:koff + ksz],
        qkT[pair][p0:p0 + 64, qoff:qoff + qsz],
        start=True, stop=True,
    )


def _av_mm(nc, v_ext, av, h, kt, src, src_off, qsz):
    koff, ksz = KT[kt]
    nc.tensor.matmul(
        av[0:128, 0:qsz],
        v_ext[kt][0:ksz, h * VW:h * VW + 128],
        src[0:ksz, src_off:src_off + qsz],
        start=(kt == 0), stop=(kt == 8),
        skip_group_check=True,
    )


_APPROX_RECIP = False


def _normalize(nc, p_sm, aoT, avs, pair, h_in_pair, qoff, qsz):
    h = 2 * pair + h_in_pair
    p0 = 64 * h_in_pair
    av_sb = p_sm.tile([128, 512], F32, tag="avsb", name="avsb")
    nc.vector.tensor_copy(av_sb[0:VW, 0:qsz], avs[h][0:VW, 0:qsz])
    rec = p_sm.tile([1, 512], F32, tag="rec", name="rec")
    if _APPROX_RECIP:
        nc.vector.reciprocal_approx_fast(rec[0:1, 0:qsz], av_sb[D:VW, 0:qsz])
    else:
        nc.vector.reciprocal(rec[0:1, 0:qsz], av_sb[D:VW, 0:qsz])
    bc = p_sm.tile([64, 512], F32, tag="bc", name="bc")
    nc.gpsimd.partition_broadcast(bc[0:64, 0:qsz], rec[0:1, 0:qsz])
    nc.vector.tensor_mul(
        aoT[pair][p0:p0 + 64, qoff:qoff + qsz],
        av_sb[0:D, 0:qsz],
        bc[0:64, 0:qsz],
    )


def _emit_qc2_head(nc, qkT, v_ext, aoT, psD, p_attn, p_sm, ATTN, pair, h_in_pair):
    qoff, qsz = QC[2]
    h = 2 * pair + h_in_pair
    sc = psD.tile([128, 1024], F32, tag="sc", name="ps_sc", bufs=3)
    for kt in range(9):
        _scores_mm(nc, qkT, sc, pair, h_in_pair, kt, kt * 8, qoff, qsz)
    a = p_attn.tile([128, 1024], ATTN, tag="attnT", name="attnT", bufs=9)
    nc.scalar.activation(
        a[:, 0:64], sc[:, 0:64],
        mybir.ActivationFunctionType.Exp, scale=SCALE,
    )
    nc.scalar.activation(
        a[0:8, 64:72], sc[0:8, 64:72],
        mybir.ActivationFunctionType.Exp, scale=SCALE,
    )
    av = psD.tile([128, 512], F32, tag="av", name="ps_av", bufs=2)
    for kt in range(9):
        _av_mm(nc, v_ext, av, h, kt, a, kt * 8, qsz)
    _normalize(nc, p_sm, aoT, {h: av}, pair, h_in_pair, qoff, qsz)


_NC_CACHE = {}
_MODE = "fp16"


def kernel(x, w_qkv, w_proj, b_proj):
    x = np.asarray(x, np.float32)
    w_qkv = np.asarray(w_qkv, np.float32)
    w_proj = np.asarray(w_proj, np.float32)
    b_proj = np.asarray(b_proj, np.float32)
    B = x.shape[0]
    assert x.shape == (8, NTOK, C), x.shape

    mt = np.float16 if _MODE == "fp16" else np.float32
    wqkT = np.ascontiguousarray(w_qkv[:2 * C].T.astype(mt))
    wvT = np.ascontiguousarray(w_qkv[2 * C:].T.astype(mt))
    wpT = np.ascontiguousarray(w_proj.T.astype(mt))
    bp = np.ascontiguousarray(b_proj.reshape(C, 1))
    in_maps = []
    for b in range(B):
        xT = np.zeros((C, T), mt)
        xT[:, :NTOK] = x[b].T.astype(mt)
        in_maps.append({"xT": xT, "wqkT": wqkT, "wvT": wvT, "wpT": wpT, "bp": bp})

    if _MODE not in _NC_CACHE:
        _NC_CACHE[_MODE] = build(matmul_dtype=_MODE)
    nc = _NC_CACHE[_MODE]
    from concourse import bass_utils
    res = bass_utils.run_bass_kernel_spmd(nc, in_maps, core_ids=list(range(B)),
                                          trace=False)
    y = np.stack([res.results[b]["yT"][:, :NTOK].T for b in range(B)])
    return np.ascontiguousarray(y.astype(np.float32))
